# revision 12
# baseline (speedup 1.0000x reference)
"""Causal self-attention (GQA + RoPE) Trainium2 Bass kernel, 8 NeuronCores.

Sharding: 2-way data parallel over batch x 4-way tensor parallel over heads.
Core c handles batch c//4 and query heads [4*(c%4), 4*(c%4)+4) plus the one
KV head g = c%4 that serves them (n_kv_heads=4 -> no KV replication).
Each core computes a partial [S, D] output (its heads' slice of the out
projection); the host sums the 4 partials per batch.

Device layouts are transposed ("feature-major"): projections produce qT/kT/vT
[dim, tokens]; attention scores are computed as S^T = kT.T @ qT.  RoPE is
handled by de-interleaving the q/k weight rows on the host so the rotation
pairs become (p, p+64) partition pairs.

v2 changes vs the 259us baseline:
 - startup: x chunk 0 is DMAd in 8 sub-pieces (2 db-blocks each) and the
   K / Q(m=0) / V projection groups are interleaved per db so the PE starts
   as soon as the first 256KB lands (~8us instead of ~16.6us).
 - softmax normalization: the denominator-reciprocal broadcast matmul is
   replaced by a stride-0 SBUF->SBUF DMA, and the psum_ot drain+scale is a
   single DVE mul reading PSUM directly (drops the PE broadcast matmul and
   the Scalar staging copy).  The per-head normalization matmul is deferred
   until two score blocks of the NEXT head are emitted, so the PE never
   head-blocks on the DVE accumulator chain (psum_ot double-buffered).
 - denominator accumulates in-place in the kb=0 e-tile (saves a DVE copy).
 - out-projection PSUM drains moved to the (otherwise idle) GpSimd engine;
   out-proj tiles are emitted interleaved between attention heads so the
   Tensor engine always has filler during exp-latency stalls.
 - the final out-projection is split into two half-head partials so its
   matmuls overlap the last attention; the host adds the extra partial.
"""

import sys

if "/opt/trn_rl_repo" not in sys.path:
    sys.path.insert(0, "/opt/trn_rl_repo")

import math

import numpy as np

D_MODEL = 2048
N_HEADS = 16
N_KV_HEADS = 4
ROPE_THETA = 10000.0
B, S = 2, 2048
DK = D_MODEL // N_HEADS          # 128
NCORES = 8
NEG = -1e30

_COMPILED = None
_TRACE = False                   # test.py flips this for profiling runs
_LAST_RESULT = None              # BassKernelResults of the last run

ATTN_ORDER = (0, 1, 3, 2)        # last one gets the split out-projection
USE_DRAM_BCAST = True            # reciprocal broadcast via DRAM bounce
USE_GPSIMD_OSB = True            # out-proj PSUM drains on GpSimd


def _build():
    import concourse.bacc as bacc
    import concourse.tile as tile
    from concourse import bass_isa, mybir

    f32 = mybir.dt.float32
    f16 = mybir.dt.float16

    nc = bacc.Bacc("TRN2", debug=False, target_bir_lowering=False)

    def inp(name, shape, dt=f16):
        return nc.declare_dram_parameter(name, list(shape), dt, isOutput=False).ap()

    x_d = inp("x", [128, 4, 16, 512])          # [part, chunk, db, tok]
    wq_d = inp("wq", [128, 4, 16, 128])        # [part, m, db, mcol]
    wkv_d = inp("wkv", [128, 2, 16, 128])      # [part, k/v, db, col]
    wc_d = inp("wc", [128, 4, 2048])
    tab_d = inp("tab", [128, 4, 2, 512])       # [part, chunk, cos/sin, tok]
    dmask_d = inp("dmask", [128, 128], f32)
    out_d = nc.declare_dram_parameter("out", [S, D_MODEL], f16, isOutput=True).ap()
    # half-head partial of the final chunk's out-projection (host adds it)
    out2_d = nc.declare_dram_parameter("out2", [512, D_MODEL], f16, isOutput=True).ap()
    # DRAM bounce buffer for the per-head reciprocal broadcast (a stride-0
    # partition read is illegal from SBUF but fine from DRAM)
    rsc_d = nc.declare_dram_parameter("rsc", [16, 512], f16, isOutput=True).ap()

    EXP = mybir.ActivationFunctionType.Exp
    SPLIT_C = ATTN_ORDER[-1]

    with tile.TileContext(nc) as tc:
        with (
            tc.tile_pool(name="consts", bufs=1) as consts,
            tc.tile_pool(name="qpool", bufs=4) as qpool,
            tc.tile_pool(name="vch", bufs=2) as vchp,
            tc.tile_pool(name="tmp", bufs=2) as tmpp,
            tc.tile_pool(name="epool", bufs=8) as epool,
            tc.tile_pool(name="accp", bufs=2) as accp,
            tc.tile_pool(name="rsum", bufs=3) as rsp,
            tc.tile_pool(name="pbp", bufs=2) as pbp,
            tc.tile_pool(name="otp", bufs=4) as otp,
            tc.tile_pool(name="osb", bufs=4) as osbp,
            tc.tile_pool(name="psum_st", bufs=2, space="PSUM") as psum_st,
            tc.tile_pool(name="psum_ot", bufs=2, space="PSUM") as psum_otp,
            tc.tile_pool(name="psum_nrm", bufs=1, space="PSUM") as psum_nrm,
            tc.tile_pool(name="psum_gen", bufs=3, space="PSUM") as psum_gen,
        ):
            # ---- constants / weights ----
            wq_sb = consts.tile([128, 4, 16, 128], f16, tag="wq")
            wkv_sb = consts.tile([128, 2, 16, 128], f16, tag="wkv")
            wc_sb = consts.tile([128, 4, 2048], f16, tag="wc")
            tab_sb = consts.tile([128, 4, 2, 512], f16, tag="tab")
            dmask_sb = consts.tile([128, 128], f32, tag="dmask")
            onescol_sb = consts.tile([128, 1], f16, tag="onescol")
            kTr_sb = consts.tile([128, S], f16, tag="kTr")
            v_sb = consts.tile([128, 16, 128], f16, tag="V")
            xT = consts.tile([128, 4, 16, 512], f16, tag="xT")
            ebias_sb = consts.tile([128, 1], f32, tag="ebias")
            nc.gpsimd.memset(ebias_sb, -2.0)
            nc.gpsimd.memset(onescol_sb, 1.0)

            # DMA plan.  Three queues (sync/gpsimd/scalar), ~140GB/s each
            # after ramp.  Chunk 0 of x goes in 8 sub-pieces alternating
            # sync/gpsimd so the first projection matmuls can start ~2us
            # after the queues open; the startup-critical weights (wkv, wq
            # m0, tab c0, dmask) go on scalar/sync in first-use order.
            nc.scalar.dma_start(out=wkv_sb[:, 0, 0:8], in_=wkv_d[:, 0, 0:8])
            nc.sync.dma_start(out=xT[:, 0, 0:2, :], in_=x_d[:, 0, 0:2, :])
            nc.gpsimd.dma_start(out=xT[:, 0, 2:4, :], in_=x_d[:, 0, 2:4, :])
            nc.scalar.dma_start(out=wkv_sb[:, 0, 8:16], in_=wkv_d[:, 0, 8:16])
            nc.sync.dma_start(out=xT[:, 0, 4:6, :], in_=x_d[:, 0, 4:6, :])
            nc.gpsimd.dma_start(out=xT[:, 0, 6:8, :], in_=x_d[:, 0, 6:8, :])
            nc.scalar.dma_start(out=wq_sb[:, 0], in_=wq_d[:, 0])
            nc.sync.dma_start(out=xT[:, 0, 8:10, :], in_=x_d[:, 0, 8:10, :])
            nc.gpsimd.dma_start(out=xT[:, 0, 10:12, :], in_=x_d[:, 0, 10:12, :])
            nc.scalar.dma_start(out=wkv_sb[:, 1, 0:8], in_=wkv_d[:, 1, 0:8])
            nc.sync.dma_start(out=xT[:, 0, 12:14, :], in_=x_d[:, 0, 12:14, :])
            nc.gpsimd.dma_start(out=xT[:, 0, 14:16, :], in_=x_d[:, 0, 14:16, :])
            nc.scalar.dma_start(out=wkv_sb[:, 1, 8:16], in_=wkv_d[:, 1, 8:16])
            nc.scalar.dma_start(out=tab_sb[:, 0], in_=tab_d[:, 0])
            nc.sync.dma_start(out=dmask_sb, in_=dmask_d)
            # chunk 1 in 4 pieces, chunks 2/3 in 2 pieces
            for j in range(4):
                q = (nc.sync, nc.gpsimd)[j % 2]
                q.dma_start(out=xT[:, 1, 4*j:4*j+4, :], in_=x_d[:, 1, 4*j:4*j+4, :])
            nc.scalar.dma_start(out=tab_sb[:, 1], in_=tab_d[:, 1])
            for m in range(1, 4):
                nc.scalar.dma_start(out=wq_sb[:, m], in_=wq_d[:, m])
            nc.sync.dma_start(out=xT[:, 2, 0:8, :], in_=x_d[:, 2, 0:8, :])
            nc.gpsimd.dma_start(out=xT[:, 2, 8:16, :], in_=x_d[:, 2, 8:16, :])
            nc.scalar.dma_start(out=tab_sb[:, 2], in_=tab_d[:, 2])
            nc.scalar.dma_start(out=tab_sb[:, 3], in_=tab_d[:, 3])
            nc.sync.dma_start(out=xT[:, 3, 0:8, :], in_=x_d[:, 3, 0:8, :])
            nc.gpsimd.dma_start(out=xT[:, 3, 8:16, :], in_=x_d[:, 3, 8:16, :])
            nc.scalar.dma_start(out=wc_sb, in_=wc_d)

            def rope(dst, src, c):
                """dst[128,512] (f16 SBUF) <- rotate(src[128,512] f32 PSUM).

                Row p<64 holds the even (te) element of pair p, row p+64 the
                odd (to): dst_lo = te*cos - to*sin; dst_hi = to*cos + te*sin.
                """
                cs = tab_sb[:, c, 0, :]
                sn = tab_sb[:, c, 1, :]
                t = tmpp.tile([128, 512], f32, tag="ropesin")
                t2 = tmpp.tile([128, 512], f32, tag="ropecos")
                nc.vector.tensor_mul(t[0:64, :], src[64:128, :], sn[0:64, :])
                nc.vector.tensor_mul(t[64:128, :], src[0:64, :], sn[64:128, :])
                nc.vector.tensor_mul(t2, src, cs)
                nc.vector.tensor_add(dst, t2, t)

            qTrs = {}
            psum = psum_gen

            def emit_proj_c(c):
                """K, Q(m=0), V projection groups for chunk c, interleaved
                per db block so all three finish right after the last x
                piece lands; then Q m=1..3 serially."""
                qTr = qpool.tile([128, 4, 512], f16, tag="qTr")
                qTrs[c] = qTr
                pk = psum.tile([128, 512], f32, tag="mm512")
                pq = psum.tile([128, 512], f32, tag="mm512")
                pv = psum.tile([128, 512], f32, tag="mm512")
                for db in range(16):
                    nc.tensor.matmul(pk, lhsT=wkv_sb[:, 0, db, :], rhs=xT[:, c, db, :],
                                     start=(db == 0), stop=(db == 15))
                    nc.tensor.matmul(pq, lhsT=wq_sb[:, 0, db, :], rhs=xT[:, c, db, :],
                                     start=(db == 0), stop=(db == 15))
                    nc.tensor.matmul(pv, lhsT=wkv_sb[:, 1, db, :], rhs=xT[:, c, db, :],
                                     start=(db == 0), stop=(db == 15))
                rope(kTr_sb[:, c * 512:(c + 1) * 512], pk, c)
                rope(qTr[:, 0, :], pq, c)
                vch = vchp.tile([128, 512], f16, tag="vch")
                nc.scalar.copy(out=vch, in_=pv)
                for rr in range(4):
                    nc.sync.dma_start_transpose(
                        out=v_sb[:, 4 * c + rr, :],
                        in_=vch[:, rr * 128:(rr + 1) * 128],
                    )
                for m in range(1, 4):
                    pq = psum.tile([128, 512], f32, tag="mm512")
                    for db in range(16):
                        nc.tensor.matmul(pq, lhsT=wq_sb[:, m, db, :],
                                         rhs=xT[:, c, db, :],
                                         start=(db == 0), stop=(db == 15))
                    rope(qTr[:, m, :], pq, c)

            # ---- out-projection, tile-granular with a pending queue ----
            outproj_pending = []   # (row, otc, tb, oc)
            _dmaq = [0]

            def queue_outproj(tq0, otc):
                for tb in range(4):
                    for oc in range(4):
                        outproj_pending.append((tq0 + tb * 128, otc, tb, oc))

            def emit_outproj_tiles(n):
                for _ in range(min(n, len(outproj_pending))):
                    row, otc, tb, oc = outproj_pending.pop(0)
                    po = psum_gen.tile([128, 512], f32, tag="mm512")
                    for h in range(4):
                        nc.tensor.matmul(
                            po,
                            lhsT=otc[:, h, tb * 128:(tb + 1) * 128],
                            rhs=wc_sb[:, h, oc * 512:(oc + 1) * 512],
                            start=(h == 0), stop=(h == 3),
                        )
                    osb = osbp.tile([128, 512], f16, tag="osb")
                    # GPSIMD cannot read PSUM -> alternate Vector/Scalar
                    if _dmaq[0] % 2 == 0:
                        nc.vector.tensor_copy(out=osb, in_=po)
                    else:
                        nc.scalar.copy(out=osb, in_=po)
                    q = (nc.sync, nc.gpsimd)[_dmaq[0] % 2]
                    _dmaq[0] += 1
                    q.dma_start(out=out_d[row:row + 128, oc * 512:(oc + 1) * 512],
                                in_=osb)

            def emit_outproj_half(tq0, otc, heads, dst, dst_row0):
                """Half-head partial out-projection -> dst (for the final
                chunk: heads 0-1 can run while heads 2-3 still attend)."""
                for tb in range(4):
                    row = tq0 + tb * 128
                    for oc in range(4):
                        po = psum_gen.tile([128, 512], f32, tag="mm512")
                        for i, h in enumerate(heads):
                            nc.tensor.matmul(
                                po,
                                lhsT=otc[:, h, tb * 128:(tb + 1) * 128],
                                rhs=wc_sb[:, h, oc * 512:(oc + 1) * 512],
                                start=(i == 0), stop=(i == len(heads) - 1),
                            )
                        osb = osbp.tile([128, 512], f16, tag="osb")
                        if _dmaq[0] % 2 == 0:
                            nc.vector.tensor_copy(out=osb, in_=po)
                        else:
                            nc.scalar.copy(out=osb, in_=po)
                        q = (nc.sync, nc.gpsimd)[_dmaq[0] % 2]
                        _dmaq[0] += 1
                        q.dma_start(
                            out=dst[row - dst_row0:row - dst_row0 + 128,
                                    oc * 512:(oc + 1) * 512],
                            in_=osb)

            def emit_attn(c, filler=False, split_out=False):
                """Attention for token chunk c, all 4 heads -> otc tile.

                The normalization chain for head h (denominator matmul,
                reciprocal, broadcast DMA, psum_ot scale) is deferred until
                two score blocks of head h+1 are in flight, so the PE never
                waits on the DVE accumulator; psum_ot is double-buffered.
                """
                nkb = 4 * c + 4
                qTr = qTrs[c]
                otc = otp.tile([128, 4, 512], f16, tag="OT")
                pending_norm = [None]

                def make_norm(h, psum_ot, acc):
                    def norm():
                        psum_sum = psum_nrm.tile([128, 512], f32, tag="nrm")
                        nc.tensor.matmul(psum_sum[0:1, :], lhsT=onescol_sb,
                                         rhs=acc, start=True, stop=True)
                        rsum = rsp.tile([1, 512], f32, tag="rsum")
                        rsumb = rsp.tile([1, 512], f16, tag="rsumb")
                        nc.vector.reciprocal_approx_fast(out=rsum, in_=psum_sum[0:1, :])
                        nc.vector.tensor_copy(out=rsumb, in_=rsum)
                        pb = pbp.tile([128, 512], f16, tag="pb")
                        slot = c * 4 + h
                        nc.sync.dma_start(out=rsc_d[slot:slot + 1, :], in_=rsumb)
                        nc.sync.dma_start(
                            out=pb,
                            in_=rsc_d[slot:slot + 1, :].to_broadcast([128, 512]))
                        nc.vector.tensor_mul(otc[:, h, :], psum_ot, pb)
                    return norm

                for h in range(4):
                    psum_ot = psum_otp.tile([128, 512], f32, tag="ot")
                    acc = None

                    def st_mm(kb):
                        rr = kb - 4 * c
                        col0 = 0 if rr < 0 else 128 * rr
                        pst = psum_st.tile([128, 512], f32, tag="st")
                        nc.tensor.matmul(
                            pst[:, col0:512],
                            lhsT=kTr_sb[:, kb * 128:(kb + 1) * 128],
                            rhs=qTr[:, h, col0:512],
                            start=True, stop=True,
                        )
                        if rr >= 0:
                            nc.vector.tensor_add(
                                pst[:, col0:col0 + 128],
                                pst[:, col0:col0 + 128],
                                dmask_sb,
                            )
                        return pst, col0

                    pending = [st_mm(0)]
                    for kb in range(nkb):
                        pst, col0 = pending.pop(0)
                        if kb + 1 < nkb:
                            pending.append(st_mm(kb + 1))
                        if kb == 0:
                            # the kb=0 e-tile doubles as the denominator
                            # accumulator, so it comes from the long-lived
                            # acc pool (epool tiles recycle every 8 blocks)
                            e = accp.tile([128, 512], f16, tag="acc")
                            acc = e
                        else:
                            e = epool.tile([128, 512], f16, tag="E")
                        nc.scalar.activation(
                            out=e[:, col0:512], in_=pst[:, col0:512], func=EXP,
                            bias=ebias_sb,
                        )
                        if kb > 0:
                            nc.vector.tensor_add(
                                acc[:, col0:512], acc[:, col0:512],
                                e[:, col0:512],
                            )
                        nc.tensor.matmul(
                            psum_ot[:, col0:512],
                            lhsT=v_sb[:, kb, :],
                            rhs=e[:, col0:512],
                            start=(kb == 0), stop=(kb == nkb - 1),
                        )
                        # finish the previous head's normalization once this
                        # head's pipeline is rolling
                        if kb == 1 and pending_norm[0] is not None:
                            pending_norm[0]()
                            pending_norm[0] = None
                        # PE filler: deferred out-projection tiles
                        if filler and kb % 2 == 1:
                            emit_outproj_tiles(1)
                    pending_norm[0] = make_norm(h, psum_ot, acc)
                # final head's normalization
                pending_norm[0]()
                return otc

            for c in range(4):
                emit_proj_c(c)

            # attn0/attn1 overlap the tail of the projection stream (which
            # is their PE filler); attn3 is filled by the out-projections of
            # chunks 0+1, attn2 by chunk 3's.  The final chunk's
            # out-projection is split into half-head partials whose first
            # half only depends on heads 0-1, letting the scheduler hoist
            # its matmuls into the last attention's exp stalls.
            for idx, c in enumerate(ATTN_ORDER):
                last = idx == len(ATTN_ORDER) - 1
                otc = emit_attn(c, filler=(idx >= 2))
                if not last:
                    queue_outproj(c * 512, otc)
                else:
                    emit_outproj_tiles(len(outproj_pending))
                    emit_outproj_half(c * 512, otc, (0, 1), out2_d, c * 512)
                    emit_outproj_half(c * 512, otc, (2, 3), out_d, 0)

    nc.compile()
    return nc


def _host_prep(x, Wq, Wkv, Wc):
    """Shard + relayout the full inputs into the 8 per-core input dicts."""
    f16 = np.float16
    dk, H, KV = DK, N_HEADS, N_KV_HEADS
    x = np.asarray(x, np.float32)
    Wq = np.asarray(Wq, np.float32)
    Wkv = np.asarray(Wkv, np.float32)
    Wc = np.asarray(Wc, np.float32)

    p = np.concatenate([np.arange(0, dk, 2), np.arange(1, dk, 2)])
    perm_q = np.concatenate([h * dk + p for h in range(H)])
    Wq_p = (Wq / math.sqrt(dk))[perm_q]
    perm_k = np.concatenate([g * dk + p for g in range(KV)])
    Wk_p = Wkv[:KV * dk][perm_k]
    Wv = Wkv[KV * dk:]

    pairs = np.arange(dk // 2, dtype=np.float64)
    freqs = 1.0 / (ROPE_THETA ** (2.0 * pairs / dk))
    ang = np.arange(S, dtype=np.float64)[:, None] * freqs[None, :]
    cos_t = np.cos(ang).astype(np.float32).T  # [64, S]
    sin_t = np.sin(ang).astype(np.float32).T
    c2 = np.concatenate([cos_t, cos_t], 0).reshape(128, 4, 512)
    ss = np.concatenate([-sin_t, sin_t], 0).reshape(128, 4, 512)
    tab = np.ascontiguousarray(np.stack([c2, ss], 2)).astype(f16)

    jj = np.arange(128)[None, :]
    pp = np.arange(128)[:, None]
    dmask = np.where(pp <= jj, 0.0, NEG).astype(np.float32)

    maps = []
    for core in range(NCORES):
        b, g = core // 4, core % 4
        wq_l = np.ascontiguousarray(
            Wq_p[512 * g:512 * g + 512].T.reshape(16, 128, 4, 128)
            .transpose(1, 2, 0, 3)
        ).astype(f16)
        wkv_sl = np.stack(
            [Wk_p[g * dk:(g + 1) * dk].T, Wv[g * dk:(g + 1) * dk].T], 0
        )  # [2, 2048, 128]
        wkv_l = np.ascontiguousarray(
            wkv_sl.reshape(2, 16, 128, 128).transpose(2, 0, 1, 3)
        ).astype(f16)
        wc_l = np.ascontiguousarray(
            Wc[:, 512 * g:512 * g + 512].T.reshape(4, 128, 2048).transpose(1, 0, 2)
        ).astype(f16)
        xt_l = np.ascontiguousarray(
            x[b].T.reshape(16, 128, 4, 512).transpose(1, 2, 0, 3)
        ).astype(f16)
        maps.append(dict(
            x=xt_l, wq=wq_l, wkv=wkv_l, wc=wc_l,
            tab=tab, dmask=dmask,
        ))
    return maps


def kernel(x, Wq, Wkv, Wc):
    global _COMPILED, _LAST_RESULT
    from concourse.bass_utils import run_bass_kernel_spmd

    if _COMPILED is None:
        _COMPILED = _build()
    in_maps = _host_prep(x, Wq, Wkv, Wc)
    res = run_bass_kernel_spmd(
        _COMPILED, in_maps, core_ids=list(range(NCORES)), trace=_TRACE
    )
    _LAST_RESULT = res
    sc = ATTN_ORDER[-1] * 512
    outs = []
    for i in range(NCORES):
        o = res.results[i]["out"].astype(np.float32)
        o[sc:sc + 512] += res.results[i]["out2"].astype(np.float32)
        outs.append(o)
    full = np.stack(
        [outs[0] + outs[1] + outs[2] + outs[3],
         outs[4] + outs[5] + outs[6] + outs[7]], 0
    )
    return full
